# revision 1
# baseline (speedup 1.0000x reference)
"""Trainium2 Bass kernel for nn_MultiGat (2-layer GAT + mean-pool + MLP).

Strategy (8 NeuronCores, SPMD single program):
  - Nodes are sharded 2500/core (padded to 2560).  Each core owns the edges
    whose *destination* lands in its node range, sorted by destination and
    grouped per 128-node destination tile (padded to G groups of 128 edges).
  - Per layer, a node "table" row [h(256) | alpha_src(4) | alpha_dst(4) | pad]
    (320 f32 = 1280 B) lives in DRAM per core (layer 1 computed redundantly;
    layer 2 distributed via AllGather of per-core shards).
  - Edge phase per core, per destination tile: dma_gather of src rows
    (1280 B) + dst alpha rows (256 B), per-edge p = exp(leaky_relu(.)) and
    messages [p*h | p] on DVE/ACT, then aggregation on the TensorEngine:
    for each group of 128 edges, a one-hot edge->dst matrix B (built on DVE
    by comparing the edge's tile-local dst id against an iota row; padding
    edges use -1 so their column is all-zero) and matmul accumulation
    out[128 dst, 260] += B^T @ msg in PSUM.  PSUM handles duplicate
    destinations natively - no scatter-add, no races.
  - Softmax uses exp without max-subtraction (values are O(1); the per-node
    normalizer cancels), with the reference's +1e-16 in the denominator.
  - Biases are folded: b is added to h pre-aggregation (softmax weights sum
    to 1), and the alpha contributions of b are host-folded constants.
  - Mean-pool partials per core via a one-hot (1/cnt) matmul; host sums the
    8 partials and runs the tiny 256->128->10 MLP in numpy.
"""

import sys

sys.path.insert(0, "/opt/trn_rl_repo")

import numpy as np


# ----------------------------------------------------------------------------
# configuration
# ----------------------------------------------------------------------------
def full_cfg():
    return dict(
        PC=8,          # cores
        NG=20000,      # global nodes
        NLOC=2500,     # nodes per core
        NPAD=2560,     # padded nodes per core (multiple of 128)
        H=4, C=64, HC=256,
        ROW=320,       # table row width (f32): h(256) as(4) ad(4) pad(56)
        G=19,          # edge groups (of 128) per destination tile
        B=32,          # graphs
        FIN=64,        # input features (pos 2 + x 62)
    )


def mini_cfg():
    return dict(
        PC=8, NG=512, NLOC=64, NPAD=128, H=4, C=64, HC=256, ROW=320,
        G=4, B=4, FIN=64,
    )


# ----------------------------------------------------------------------------
# device program
# ----------------------------------------------------------------------------

def _patch_tile_swdge_lane_by_queue():
    """Pin each Pool-engine DMA instruction's DMASW sem lane to its SWDGE
    queue_num (Tile's default round-robin mixes queues on one sem lane,
    which the scheduler rejects when num_swdge_queues > 1)."""
    import concourse.tile_sem_assignment as tsa
    if getattr(tsa, "_lane_by_queue_patched", False):
        return
    tsa._lane_by_queue_patched = True
    import concourse.mybir as mybir
    import concourse.bass_isa as bass_isa

    orig = tsa.TileClockTick._assign_tick

    def _assign_tick(self, inst):
        from concourse.tile_scheduler import DMAInst
        if (
            isinstance(inst, DMAInst)
            and not isinstance(inst, bass_isa.UserSyncedRemoteDMADescs)
            and inst.engine == mybir.EngineType.Pool
        ):
            q = int(getattr(inst, "queue_num", 0) or 0)
            self.next_sw_dma_idx = q
        return orig(self, inst)

    tsa.TileClockTick._assign_tick = _assign_tick


def build_program(cfg, reps=1):
    import concourse.mybir as mybir
    import concourse.bacc as bacc
    import concourse.tile as tile

    f32 = mybir.dt.float32
    i16 = mybir.dt.int16
    AF = mybir.ActivationFunctionType

    PC, NPAD, ROW, HC, H, C, G = (
        cfg["PC"], cfg["NPAD"], cfg["ROW"], cfg["HC"], cfg["H"], cfg["C"],
        cfg["G"])
    B, FIN = cfg["B"], cfg["FIN"]
    NTBL = PC * NPAD           # table rows (global, padded)
    NT_T = NTBL // 128         # node tiles for table build
    NL_T = NPAD // 128         # local node tiles (= edge chunks per layer)
    CH = G * 128               # tokens per chunk (one dst tile)
    EPAD = CH * NL_T           # padded edge tokens per core
    ICOLS = EPAD // 16
    NAUG = HC + H              # aggregated row: [msg(256) | p(4)]

    _patch_tile_swdge_lane_by_queue()
    NQ = cfg.get("NQ", 4)
    nc = bacc.Bacc(None, target_bir_lowering=False, debug=True,
                   num_swdge_queues=NQ)

    # ---- I/O
    xt = nc.declare_dram_parameter("xt", [FIN, NTBL], f32, isOutput=False)
    w1 = nc.declare_dram_parameter("w1", [FIN, ROW], f32, isOutput=False)
    b1f = nc.declare_dram_parameter("b1f", [128, ROW], f32, isOutput=False)
    w2 = nc.declare_dram_parameter("w2", [128, 2, ROW], f32, isOutput=False)
    b2f = nc.declare_dram_parameter("b2f", [128, ROW], f32, isOutput=False)
    ident = nc.declare_dram_parameter("ident", [128, 128], f32, isOutput=False)
    iota = nc.declare_dram_parameter("iota", [128, 128], f32, isOutput=False)
    srcw = nc.declare_dram_parameter("srcw", [128, ICOLS], i16, isOutput=False)
    gdstw = nc.declare_dram_parameter("gdstw", [128, ICOLS], i16, isOutput=False)
    bloc = nc.declare_dram_parameter("bloc", [128, NL_T * G], f32, isOutput=False)
    mpool = nc.declare_dram_parameter("mpool", [128, NL_T, B], f32, isOutput=False)
    pooled = nc.declare_dram_parameter("pooled", [B, HC], f32, isOutput=True)

    # ---- internal DRAM
    T1 = nc.dram_tensor("T1", [NTBL, ROW], f32)
    T2s = nc.dram_tensor("T2s", [NPAD, ROW], f32)
    T2 = nc.dram_tensor("T2", [NTBL, ROW], f32, addr_space="Shared")

    with tile.TileContext(nc) as tc:
        with tc.tile_pool(name="persist", bufs=1) as pp:
            si = pp.tile([128, ICOLS], i16)
            gi = pp.tile([128, ICOLS], i16)
            bl = pp.tile([128, NL_T * G], f32)
            io = pp.tile([128, 128], f32)
            nc.sync.dma_start(si[:], srcw[:])
            nc.sync.dma_start(gi[:], gdstw[:])
            nc.sync.dma_start(bl[:], bloc[:])
            nc.sync.dma_start(io[:], iota[:])

            for _rep in range(reps):
                # ------------- phase 0: build T1 (replicated) -------------
                with (
                    tc.tile_pool(name="p0", bufs=3) as p0,
                    tc.tile_pool(name="p0w", bufs=1) as p0w,
                    tc.tile_pool(name="ps0", bufs=4, space="PSUM") as ps0,
                ):
                    xts = p0w.tile([FIN, NTBL], f32)
                    nc.sync.dma_start(xts[:], xt[:])
                    w1s = p0w.tile([FIN, ROW], f32)
                    nc.sync.dma_start(w1s[:], w1[:])
                    b1s = p0w.tile([128, ROW], f32)
                    nc.sync.dma_start(b1s[:], b1f[:])
                    for j in range(NT_T):
                        j0 = j * 128
                        ps = ps0.tile([128, ROW], f32)
                        nc.tensor.matmul(ps[:], xts[:, j0:j0 + 128], w1s[:],
                                         start=True, stop=True)
                        ts = p0.tile([128, ROW], f32)
                        nc.vector.tensor_add(ts[:], ps[:], b1s[:])
                        nc.sync.dma_start(T1[j0:j0 + 128, :], ts[:])

                # ------------- edge phase: one dst tile per chunk ----------
                # consume(j, o) receives the normalized output tile
                # o [128, HC] for local node tile j.
                def edge_phase(T, tag, consume):
                    if cfg.get("SKIP_EDGES"):
                        for k in range(NL_T):
                            with tc.tile_pool(name=f"z{tag}", bufs=1) as zp:
                                o = zp.tile([128, HC], f32)
                                nc.vector.memset(o[:], 0.0)
                                consume(k, o)
                        return
                    with (
                        tc.tile_pool(name=f"e{tag}", bufs=2) as ep,
                        tc.tile_pool(name=f"es{tag}", bufs=3) as esp,
                        tc.tile_pool(name=f"eps{tag}", bufs=2,
                                     space="PSUM") as epsp,
                    ):
                        for k in range(NL_T):
                            cols = slice(k * (CH // 16), (k + 1) * (CH // 16))
                            g1 = ep.tile([128, G, ROW], f32, tag="g1")
                            nc.gpsimd.dma_gather(
                                g1[:], T[:, :], si[:, cols], CH, CH, ROW,
                                elem_step=ROW, single_packet=False,
                                queue_num=k % 2 if NQ > 1 else 0)
                            g2 = ep.tile([128, G, 64], f32, tag="g2")
                            nc.gpsimd.dma_gather(
                                g2[:], T[:, HC:HC + 64], gi[:, cols], CH, CH,
                                64, elem_step=ROW, single_packet=False,
                                queue_num=(2 + k % 2) if NQ > 1 else 0)
                            se = esp.tile([128, G, H], f32, tag="se")
                            nc.vector.tensor_add(
                                se[:], g1[:, :, HC:HC + H], g2[:, :, H:2 * H])
                            lr = esp.tile([128, G, H], f32, tag="lr")
                            nc.vector.tensor_scalar_mul(lr[:], se[:], 0.2)
                            lr2 = esp.tile([128, G, H], f32, tag="lr2")
                            nc.vector.tensor_max(lr2[:], se[:], lr[:])
                            mp = ep.tile([128, G, NAUG], f32, tag="mp")
                            nc.scalar.activation(
                                mp[:, :, HC:HC + H], lr2[:], AF.Exp)
                            pv = mp[:, :, HC:HC + H]
                            if cfg.get("SKIP_MUL"):
                                nc.vector.tensor_copy(
                                    mp[:, :, 0:HC], g1[:, :, 0:HC])
                            else:
                                pb = pv.unsqueeze(3).broadcast_to(
                                    [128, G, H, C])
                                nc.vector.tensor_mul(
                                    mp[:, :, 0:HC].rearrange(
                                        "p m (h c) -> p m h c", c=C),
                                    g1[:, :, 0:HC].rearrange(
                                        "p m (h c) -> p m h c", c=C),
                                    pb)
                            # aggregate via one-hot matmuls
                            acc = epsp.tile([128, NAUG], f32, tag="acc")
                            if cfg.get("SKIP_AGG"):
                                nc.vector.memset(acc[:], 1.0)
                            else:
                                for g in range(G):
                                    bt = esp.tile([128, 128], f32, tag="bt")
                                    nc.vector.tensor_scalar(
                                        bt[:], io[:],
                                        bl[:, k * G + g:k * G + g + 1],
                                        None, mybir.AluOpType.is_equal)
                                    nc.tensor.matmul(
                                        acc[:], bt[:], mp[:, g, :],
                                        start=(g == 0), stop=(g == G - 1))
                            # normalize: o = num / (den + 1e-16)
                            nc.vector.tensor_scalar_add(
                                acc[:, HC:HC + H], acc[:, HC:HC + H], 1e-16)
                            rd = esp.tile([128, H], f32, tag="rd")
                            nc.vector.reciprocal(rd[:], acc[:, HC:HC + H])
                            o = esp.tile([128, HC], f32, tag="o")
                            for h in range(H):
                                nc.vector.tensor_scalar_mul(
                                    o[:, h * C:(h + 1) * C],
                                    acc[:, h * C:(h + 1) * C], rd[:, h:h + 1])
                            consume(k, o)

                # ------------- layer 1 + transpose into o1T -------------
                with (
                    tc.tile_pool(name="p2w", bufs=1) as p2w,
                    tc.tile_pool(name="pst", bufs=4, space="PSUM") as pst,
                ):
                    ids = p2w.tile([128, 128], f32)
                    nc.sync.dma_start(ids[:], ident[:])
                    o1T = p2w.tile([128, 2, NPAD], f32)

                    def consume1(j, o):
                        j0 = j * 128
                        for kk in range(2):
                            pt = pst.tile([128, 128], f32, tag="pt")
                            nc.tensor.transpose(
                                pt[:], o[:, kk * 128:(kk + 1) * 128], ids[:])
                            nc.vector.tensor_copy(
                                o1T[:, kk, j0:j0 + 128], pt[:])

                    edge_phase(T1, "1", consume1)

                    # ------------- T2 shard + AllGather -------------
                    with (
                        tc.tile_pool(name="p2", bufs=3) as p2,
                        tc.tile_pool(name="ps2", bufs=4, space="PSUM") as ps2,
                    ):
                        w2s = p2w.tile([128, 2, ROW], f32)
                        nc.sync.dma_start(w2s[:], w2[:])
                        b2s = p2w.tile([128, ROW], f32)
                        nc.sync.dma_start(b2s[:], b2f[:])
                        for j in range(NL_T):
                            j0 = j * 128
                            ps = ps2.tile([128, ROW], f32, tag="mm")
                            nc.tensor.matmul(ps[:], o1T[:, 0, j0:j0 + 128],
                                             w2s[:, 0, :],
                                             start=True, stop=False)
                            nc.tensor.matmul(ps[:], o1T[:, 1, j0:j0 + 128],
                                             w2s[:, 1, :],
                                             start=False, stop=True)
                            ts = p2.tile([128, ROW], f32, tag="t2row")
                            nc.vector.tensor_add(ts[:], ps[:], b2s[:])
                            nc.sync.dma_start(T2s[j0:j0 + 128, :], ts[:])

                        nc.gpsimd.collective_compute(
                            "AllGather",
                            mybir.AluOpType.bypass,
                            replica_groups=[list(range(PC))],
                            ins=[T2s[:]],
                            outs=[T2[:]],
                        )

                # ------------- layer 2 + pooling -------------
                with (
                    tc.tile_pool(name="p4w", bufs=1) as p4w,
                    tc.tile_pool(name="ps4", bufs=2, space="PSUM") as ps4,
                ):
                    o2buf = p4w.tile([128, NL_T, HC], f32)

                    def consume2(j, o):
                        nc.vector.tensor_copy(o2buf[:, j, :], o[:])

                    edge_phase(T2, "2", consume2)

                    mps = p4w.tile([128, NL_T, B], f32)
                    nc.sync.dma_start(mps[:], mpool[:])
                    acc = ps4.tile([B, HC], f32)
                    for j in range(NL_T):
                        nc.tensor.matmul(acc[:], mps[:, j, :], o2buf[:, j, :],
                                         start=(j == 0), stop=(j == NL_T - 1))
                    po = p4w.tile([B, HC], f32)
                    nc.vector.tensor_copy(po[:], acc[:])
                    nc.sync.dma_start(pooled[:], po[:])

        _, _snap = tc.schedule_and_allocate()
        nc.predicted_ns = _snap.time if _snap is not None else None

    nc.compile()
    return nc


# ----------------------------------------------------------------------------
# host-side preparation
# ----------------------------------------------------------------------------
def pack_edges(cfg, src_g, dst_g, core):
    """Sort this core's edges by destination, group per 128-node dst tile,
    pad each tile's run to G*128 tokens.  Returns (src_idx, gdst_idx, bloc)
    where bloc[t] is the tile-local dst id (0..127) or -1 for padding."""
    NLOC, NPAD, G = cfg["NLOC"], cfg["NPAD"], cfg["G"]
    NL_T = NPAD // 128
    CH = G * 128
    EPAD = CH * NL_T
    lo = core * NLOC
    sel = (dst_g >= lo) & (dst_g < lo + NLOC)
    es = src_g[sel]
    ed = dst_g[sel] - lo
    order = np.argsort(ed, kind="stable")
    es, ed = es[order], ed[order]

    src_idx = np.zeros(EPAD, dtype=np.int16)
    gdst_idx = np.zeros(EPAD, dtype=np.int16)
    bloc = np.full(EPAD, -1.0, dtype=np.float32)
    remap = lambda gidx: (gidx // NLOC) * NPAD + (gidx % NLOC)
    tile_of = ed // 128
    starts = np.searchsorted(tile_of, np.arange(NL_T), side="left")
    ends = np.searchsorted(tile_of, np.arange(NL_T), side="right")
    for t in range(NL_T):
        a, b = starts[t], ends[t]
        cnt = b - a
        assert cnt <= CH, f"dst tile {t} has {cnt} edges > capacity {CH}"
        p0 = t * CH
        src_idx[p0:p0 + cnt] = remap(es[a:b]).astype(np.int16)
        gdst_idx[p0:p0 + cnt] = remap(ed[a:b] + lo).astype(np.int16)
        bloc[p0:p0 + cnt] = (ed[a:b] - t * 128).astype(np.float32)
    return src_idx, gdst_idx, bloc


def wrap16(idx):
    """[EPAD] token array -> [128, EPAD/16] wrapped+replicated layout."""
    w = idx.reshape(-1, 16).T  # [16, EPAD/16]
    return np.ascontiguousarray(np.tile(w, (8, 1)))


def wrap128(vals):
    """[EPAD] token array -> [128, EPAD/128] (token t at [t%128, t//128])."""
    return np.ascontiguousarray(vals.reshape(-1, 128).T)


def host_prepare(cfg, x, pos, edge_index, batch,
                 W1, a_src1, a_dst1, b1, W2, a_src2, a_dst2, b2):
    PC, NG, NLOC, NPAD, H, C, HC, FIN, B = (
        cfg["PC"], cfg["NG"], cfg["NLOC"], cfg["NPAD"], cfg["H"], cfg["C"],
        cfg["HC"], cfg["FIN"], cfg["B"])
    NTBL = PC * NPAD

    x_in = np.concatenate([pos, x], axis=1).astype(np.float32)  # [NG, FIN]
    loop = np.arange(NG, dtype=np.int64)
    src = np.concatenate([np.asarray(edge_index[0]), loop])
    dst = np.concatenate([np.asarray(edge_index[1]), loop])

    xpad = np.zeros((NTBL, FIN), np.float32)
    for c in range(PC):
        xpad[c * NPAD:c * NPAD + NLOC] = x_in[c * NLOC:(c + 1) * NLOC]
    xt = np.ascontiguousarray(xpad.T)

    def augment(W, a_s, a_d, b):
        ROW = cfg["ROW"]
        wad = np.einsum("fhc,hc->fh", W.reshape(W.shape[0], H, C), a_d)
        was = np.einsum("fhc,hc->fh", W.reshape(W.shape[0], H, C), a_s)
        waug = np.concatenate(
            [W, was, wad,
             np.zeros((W.shape[0], ROW - HC - 2 * H), np.float32)],
            axis=1).astype(np.float32)
        cs = np.einsum("hc,hc->h", b.reshape(H, C), a_s)
        cd = np.einsum("hc,hc->h", b.reshape(H, C), a_d)
        brow = np.concatenate(
            [b, cs, cd,
             np.zeros(ROW - HC - 2 * H, np.float32)]).astype(np.float32)
        return waug, brow

    w1aug, b1row = augment(W1, a_src1, a_dst1, b1)
    w2aug, b2row = augment(W2, a_src2, a_dst2, b2)
    b1f = np.ascontiguousarray(np.broadcast_to(b1row, (128, b1row.shape[0])))
    b2f = np.ascontiguousarray(np.broadcast_to(b2row, (128, b2row.shape[0])))
    w2k = np.ascontiguousarray(
        w2aug.reshape(2, 128, w2aug.shape[1]).transpose(1, 0, 2))
    ident = np.eye(128, dtype=np.float32)
    iota = np.ascontiguousarray(
        np.broadcast_to(np.arange(128, dtype=np.float32), (128, 128)))

    cnt = np.bincount(np.asarray(batch).astype(np.int64), minlength=B)
    in_maps = []
    for c in range(PC):
        si, gi, blv = pack_edges(cfg, src, dst, c)
        mp = np.zeros((NPAD, B), np.float32)
        gb = np.asarray(batch)[c * NLOC:(c + 1) * NLOC].astype(np.int64)
        mp[np.arange(NLOC), gb] = 1.0 / np.maximum(cnt[gb], 1.0)
        mpool = np.ascontiguousarray(
            mp.reshape(NPAD // 128, 128, B).transpose(1, 0, 2))
        in_maps.append(dict(
            xt=xt, w1=w1aug, b1f=b1f, w2=w2k, b2f=b2f, ident=ident, iota=iota,
            srcw=wrap16(si), gdstw=wrap16(gi), bloc=wrap128(blv), mpool=mpool,
        ))
    return in_maps


def host_tail(pooled_parts, lw1, lb1, lw2, lb2):
    pooled = np.sum(np.stack(pooled_parts), axis=0)
    y = np.maximum(pooled @ lw1 + lb1, 0.0)
    y = np.maximum(y @ lw2 + lb2, 0.0)
    return y.astype(np.float32)


# ----------------------------------------------------------------------------
# entry point
# ----------------------------------------------------------------------------
_CACHE = {}


def kernel(**inputs):
    from concourse.bass_utils import run_bass_kernel_spmd

    cfg = full_cfg()
    inp = {k: np.asarray(v) for k, v in inputs.items()}
    in_maps = host_prepare(
        cfg, inp["x"], inp["pos"], inp["edge_index"], inp["batch"],
        inp["W1"], inp["a_src1"], inp["a_dst1"], inp["b1"],
        inp["W2"], inp["a_src2"], inp["a_dst2"], inp["b2"])
    if "nc" not in _CACHE:
        _CACHE["nc"] = build_program(cfg)
    nc = _CACHE["nc"]
    res = run_bass_kernel_spmd(nc, in_maps, list(range(cfg["PC"])))
    parts = [res.results[c]["pooled"] for c in range(cfg["PC"])]
    return host_tail(parts, inp["lw1"], inp["lb1"], inp["lw2"], inp["lb2"])



# revision 9
# speedup vs baseline: 2.0277x; 2.0277x over previous
"""Trainium2 Bass kernel for nn_MultiGat (2-layer GAT + mean-pool + MLP).

Strategy (8 NeuronCores, SPMD single program), v2:
  - Nodes sharded 2500/core (padded 2560).  Each core owns the edges whose
    destination lands in its range, grouped per 128-node destination tile
    (chunk), padded to G groups of 128 edge tokens per chunk.
  - Table row per node (bf16, ROW=384 cols = 768 B): [h+b (256) | a_src (4)
    | a_dst (4) | pad].  Layer-1 table replicated per core; layer-2 table
    sharded + AllGather (as in the reference sharding hint).
  - Per chunk ONE dma_gather fetches the full src row per edge token
    (768 B, one descriptor per token).  Padding tokens use trailing -1
    indices, which the SWDGE Q7 kernel truncates before descriptor
    generation (chunks 0/1 of each layer pad with row 0 instead, because
    their SBUF buffers hold uninitialized bits on first use).
  - Token 0..127 of each chunk are the 128 destination nodes' self-loop
    edges in destination order, so the gathered group-0 rows double as the
    per-destination a_dst table ([128, H] aligned by partition) -- no
    second gather and no per-core dynamic addressing.
  - The edge->dst one-hot (bt) and its transpose (btT) are STATIC graph
    structure: host-precomputed bf16 DRAM tensors streamed by regular DMA
    (no Q7 descriptor cost, no per-group DVE is_equal builds).  btT gives
    per-edge a_dst via tiny matmuls adp_g = btT_g^T @ ad_tile; bt
    aggregates messages acc += bt_g^T @ [p*h | p] in PSUM (duplicate
    destinations accumulate natively).
  - Softmax uses exp without max-subtraction (values are O(1); the
    per-node normalizer cancels), with the reference's +1e-16 in the
    denominator.  leaky_relu and exp run on the Scalar engine.
  - Biases are folded: b is added to h during the table build via a
    ones-row in the stationary operand (softmax weights sum to 1), and the
    alpha contributions of b are host-folded into that bias row.
  - Mean-pool partials per core via a (1/cnt) matmul; host sums the 8
    partials and runs the tiny 256->128->10 MLP in numpy.
"""

import sys

sys.path.insert(0, "/opt/trn_rl_repo")

import numpy as np
import ml_dtypes


# ----------------------------------------------------------------------------
# configuration
# ----------------------------------------------------------------------------
def full_cfg():
    return dict(
        PC=8,          # cores
        NG=20000,      # global nodes
        NLOC=2500,     # nodes per core
        NPAD=2560,     # padded nodes per core (multiple of 128)
        H=4, C=64, HC=256,
        ROW=384,       # table row width (bf16): h(256) as(4) ad(4) pad(120)
        G=19,          # edge groups (of 128) per destination tile
        B=32,          # graphs
        FIN=64,        # input features (pos 2 + x 62)
    )


# ----------------------------------------------------------------------------
# device program
# ----------------------------------------------------------------------------

def _patch_tile_swdge_lane_by_queue():
    """Pin each Pool-engine DMA instruction's DMASW sem lane to its SWDGE
    queue_num (Tile's default round-robin mixes queues on one sem lane,
    which the scheduler rejects when num_swdge_queues > 1)."""
    import concourse.tile_sem_assignment as tsa
    if getattr(tsa, "_lane_by_queue_patched", False):
        return
    tsa._lane_by_queue_patched = True
    import concourse.mybir as mybir
    import concourse.bass_isa as bass_isa

    orig = tsa.TileClockTick._assign_tick

    def _assign_tick(self, inst):
        from concourse.tile_scheduler import DMAInst
        if (
            isinstance(inst, DMAInst)
            and not isinstance(inst, bass_isa.UserSyncedRemoteDMADescs)
            and inst.engine == mybir.EngineType.Pool
        ):
            q = int(getattr(inst, "queue_num", 0) or 0)
            self.next_sw_dma_idx = q
        return orig(self, inst)

    tsa.TileClockTick._assign_tick = _assign_tick


def build_program(cfg, reps=1):
    import concourse.mybir as mybir
    import concourse.bacc as bacc
    import concourse.tile as tile

    f32 = mybir.dt.float32
    bf16 = mybir.dt.bfloat16
    i16 = mybir.dt.int16
    AF = mybir.ActivationFunctionType

    PC, NPAD, ROW, HC, H, C, G = (
        cfg["PC"], cfg["NPAD"], cfg["ROW"], cfg["HC"], cfg["H"], cfg["C"],
        cfg["G"])
    B, FIN = cfg["B"], cfg["FIN"]
    NTBL = PC * NPAD           # table rows (global, padded)
    NT_T = NTBL // 128         # node tiles for table build
    NL_T = NPAD // 128         # local node tiles (= edge chunks per layer)
    CH = G * 128               # tokens per chunk (one dst tile)
    EPAD = CH * NL_T           # padded edge tokens per core
    ICOLS = EPAD // 16
    NAUG = HC + H              # aggregated row: [msg(256) | p(4)]
    ACT_W = HC + 2 * H         # active row columns: h | a_src | a_dst

    _patch_tile_swdge_lane_by_queue()
    NQ = cfg.get("NQ", 4)
    nc = bacc.Bacc(None, target_bir_lowering=False, debug=True,
                   num_swdge_queues=NQ)

    # ---- I/O
    # xt carries an extra all-ones row (row FIN) so the table matmul adds
    # w1's bias row directly.
    xt = nc.declare_dram_parameter("xt", [FIN + 1, NTBL], bf16, isOutput=False)
    w1 = nc.declare_dram_parameter("w1", [FIN + 1, ACT_W], bf16,
                                   isOutput=False)
    w2 = nc.declare_dram_parameter("w2", [128, 2, ACT_W], bf16,
                                   isOutput=False)
    b2r = nc.declare_dram_parameter("b2r", [1, ACT_W], bf16, isOutput=False)
    ones1 = nc.declare_dram_parameter("ones1", [1, 128], bf16, isOutput=False)
    ident = nc.declare_dram_parameter("ident", [128, 128], f32,
                                      isOutput=False)
    srcw = nc.declare_dram_parameter("srcw", [128, ICOLS], i16, isOutput=False)
    btd = nc.declare_dram_parameter("btd", [NL_T, 128, CH], bf16,
                                    isOutput=False)
    btTd = nc.declare_dram_parameter("btTd", [NL_T, 128, CH], bf16,
                                     isOutput=False)
    mpool = nc.declare_dram_parameter("mpool", [128, NL_T, B], bf16,
                                      isOutput=False)
    pooled = nc.declare_dram_parameter("pooled", [B, HC], f32, isOutput=True)

    # ---- internal DRAM
    T1 = nc.dram_tensor("T1", [NTBL, ROW], bf16)
    T2s = nc.dram_tensor("T2s", [NPAD, ROW], bf16)
    T2 = nc.dram_tensor("T2", [NTBL, ROW], bf16, addr_space="Shared")

    with tile.TileContext(nc) as tc:
        with tc.tile_pool(name="persist", bufs=1) as pp:
            si = pp.tile([128, ICOLS], i16)
            on1 = pp.tile([1, 128], bf16)
            ids = pp.tile([128, 128], f32)
            nc.sync.dma_start(si[:], srcw[:])
            nc.sync.dma_start(on1[:], ones1[:])
            nc.sync.dma_start(ids[:], ident[:])

            for _rep in range(reps):
                # ------------- phase 0: build T1 (replicated) -------------
                with (
                    tc.tile_pool(name="p0", bufs=3) as p0,
                    tc.tile_pool(name="p0w", bufs=1) as p0w,
                    tc.tile_pool(name="ps0", bufs=4, space="PSUM") as ps0,
                ):
                    xts = p0w.tile([FIN + 1, NTBL], bf16)
                    nc.sync.dma_start(xts[:], xt[:])
                    w1s = p0w.tile([FIN + 1, ACT_W], bf16)
                    nc.sync.dma_start(w1s[:], w1[:])
                    for j in range(NT_T):
                        j0 = j * 128
                        ps = ps0.tile([128, ACT_W], f32)
                        nc.tensor.matmul(ps[:], xts[:, j0:j0 + 128], w1s[:],
                                         start=True, stop=True)
                        ts = p0.tile([128, ACT_W], bf16)
                        nc.scalar.activation(ts[:], ps[:], AF.Copy)
                        nc.sync.dma_start(T1[j0:j0 + 128, 0:ACT_W], ts[:])

                # ------------- edge phase: one dst tile per chunk ----------
                # consume(j, o) receives the normalized output tile
                # o [128, HC] (bf16) for local node tile j.
                def edge_phase(T, tag, consume):
                    with (
                        tc.tile_pool(name=f"e{tag}", bufs=2) as ep,
                        tc.tile_pool(name=f"eb{tag}", bufs=2) as ebp,
                        tc.tile_pool(name=f"es{tag}", bufs=3) as esp,
                        tc.tile_pool(name=f"eps{tag}", bufs=2,
                                     space="PSUM") as epsp,
                        tc.tile_pool(name=f"aps{tag}", bufs=2,
                                     space="PSUM") as apsp,
                    ):
                        for k in range(NL_T):
                            cols = slice(k * (CH // 16), (k + 1) * (CH // 16))
                            bts = ebp.tile([128, G, 128], bf16, tag="bt")
                            nc.sync.dma_start(
                                bts[:],
                                btd[k].rearrange("p (g d) -> p g d", g=G))
                            btTs = ebp.tile([128, G, 128], bf16, tag="btT")
                            nc.sync.dma_start(
                                btTs[:],
                                btTd[k].rearrange("p (g e) -> p g e", g=G))
                            g1 = ep.tile([128, G, ROW], bf16, tag="g1")
                            nc.gpsimd.dma_gather(
                                g1[:], T[:, :], si[:, cols], CH, CH, ROW,
                                elem_step=ROW, single_packet=False,
                                queue_num=k % NQ)
                            # per-edge a_dst via one-hot-transpose matmuls;
                            # the ad table is the gathered self-loop rows.
                            adt = g1[:, 0, HC + H:HC + 2 * H]
                            adp = apsp.tile([128, G, H], f32, tag="adp")
                            for g in range(G):
                                nc.tensor.matmul(
                                    adp[:, g, :], btTs[:, g, :], adt,
                                    start=True, stop=True)
                            se = esp.tile([128, G, H], f32, tag="se")
                            nc.vector.tensor_add(
                                se[:], g1[:, :, HC:HC + H], adp[:])
                            lr = esp.tile([128, G, H], f32, tag="lr")
                            nc.vector.tensor_scalar_mul(lr[:], se[:], 0.2)
                            lr2 = esp.tile([128, G, H], f32, tag="lr2")
                            nc.vector.tensor_max(lr2[:], se[:], lr[:])
                            mp = ep.tile([128, G, NAUG], bf16, tag="mp")
                            pv = mp[:, :, HC:HC + H]
                            nc.scalar.activation(pv, lr2[:], AF.Exp)
                            pb = pv.unsqueeze(3).broadcast_to([128, G, H, C])
                            nc.vector.tensor_mul(
                                mp[:, :, 0:HC].rearrange(
                                    "p m (h c) -> p m h c", c=C),
                                g1[:, :, 0:HC].rearrange(
                                    "p m (h c) -> p m h c", c=C),
                                pb)
                            # aggregate via one-hot matmuls
                            acc = epsp.tile([128, NAUG], f32, tag="acc")
                            for g in range(G):
                                nc.tensor.matmul(
                                    acc[:], bts[:, g, :], mp[:, g, :],
                                    start=(g == 0), stop=(g == G - 1))
                            # normalize: o = num / (den + 1e-16)
                            nc.vector.tensor_scalar_add(
                                acc[:, HC:HC + H], acc[:, HC:HC + H], 1e-16)
                            rd = esp.tile([128, H], f32, tag="rd")
                            nc.vector.reciprocal(rd[:], acc[:, HC:HC + H])
                            o = esp.tile([128, HC], f32, tag="o")
                            rb = rd[:].unsqueeze(2).broadcast_to([128, H, C])
                            nc.vector.tensor_mul(
                                o[:].rearrange("p (h c) -> p h c", c=C),
                                acc[:, 0:HC].rearrange(
                                    "p (h c) -> p h c", c=C),
                                rb)
                            consume(k, o)

                # ------------- layer 1 + transpose into o1T -------------
                with (
                    tc.tile_pool(name="p2w", bufs=1) as p2w,
                    tc.tile_pool(name="pst", bufs=4, space="PSUM") as pst,
                ):
                    o1T = p2w.tile([128, 2, NPAD], bf16)

                    def consume1(j, o):
                        j0 = j * 128
                        for kk in range(2):
                            pt = pst.tile([128, 128], f32, tag="pt")
                            nc.tensor.transpose(
                                pt[:], o[:, kk * 128:(kk + 1) * 128], ids[:])
                            nc.vector.tensor_copy(
                                o1T[:, kk, j0:j0 + 128], pt[:])

                    edge_phase(T1, "1", consume1)

                    # ------------- T2 shard + AllGather -------------
                    with (
                        tc.tile_pool(name="p2", bufs=3) as p2,
                        tc.tile_pool(name="ps2", bufs=4, space="PSUM") as ps2,
                    ):
                        w2s = p2w.tile([128, 2, ACT_W], bf16)
                        nc.sync.dma_start(w2s[:], w2[:])
                        b2s = p2w.tile([1, ACT_W], bf16)
                        nc.sync.dma_start(b2s[:], b2r[:])
                        for j in range(NL_T):
                            j0 = j * 128
                            ps = ps2.tile([128, ACT_W], f32, tag="mm")
                            nc.tensor.matmul(ps[:], o1T[:, 0, j0:j0 + 128],
                                             w2s[:, 0, :],
                                             start=True, stop=False)
                            nc.tensor.matmul(ps[:], o1T[:, 1, j0:j0 + 128],
                                             w2s[:, 1, :],
                                             start=False, stop=False)
                            nc.tensor.matmul(ps[:], on1[:], b2s[:],
                                             start=False, stop=True)
                            ts = p2.tile([128, ACT_W], bf16, tag="t2row")
                            nc.scalar.activation(ts[:], ps[:], AF.Copy)
                            nc.sync.dma_start(T2s[j0:j0 + 128, 0:ACT_W],
                                              ts[:])

                        nc.gpsimd.collective_compute(
                            "AllGather",
                            mybir.AluOpType.bypass,
                            replica_groups=[list(range(PC))],
                            ins=[T2s[:].bitcast(f32)],
                            outs=[T2[:].bitcast(f32)],
                        )

                # ------------- layer 2 + pooling -------------
                with (
                    tc.tile_pool(name="p4w", bufs=1) as p4w,
                    tc.tile_pool(name="ps4", bufs=2, space="PSUM") as ps4,
                ):
                    o2buf = p4w.tile([128, NL_T, HC], bf16)

                    def consume2(j, o):
                        nc.vector.tensor_copy(o2buf[:, j, :], o[:])

                    edge_phase(T2, "2", consume2)

                    mps = p4w.tile([128, NL_T, B], bf16)
                    nc.sync.dma_start(mps[:], mpool[:])
                    acc = ps4.tile([B, HC], f32)
                    for j in range(NL_T):
                        nc.tensor.matmul(acc[:], mps[:, j, :], o2buf[:, j, :],
                                         start=(j == 0), stop=(j == NL_T - 1))
                    po = p4w.tile([B, HC], f32)
                    nc.vector.tensor_copy(po[:], acc[:])
                    nc.sync.dma_start(pooled[:], po[:])

        _, _snap = tc.schedule_and_allocate()
        nc.predicted_ns = _snap.time if _snap is not None else None

    nc.compile()
    return nc


# ----------------------------------------------------------------------------
# host-side preparation
# ----------------------------------------------------------------------------
def pack_edges(cfg, src_g, dst_g, core):
    """Build this core's edge-token stream.  Per 128-dst tile (chunk):
    tokens 0..127 are the tile's self-loop edges in destination order
    (token d = self loop of local dst d, so the gathered group-0 rows
    serve as the per-destination a_dst table); tokens 128.. are the core's
    incident random edges sorted by destination.  Padding uses -1 (Q7
    drops trailing negatives) except chunks 0/1 and non-trailing slots,
    which point at row 0.  Returns (src_idx [EPAD] int16,
    bt [NL_T, CH, 128], btT [NL_T, 128, CH]) with bt[k, t, d] = 1 iff
    token t of chunk k targets local dst d."""
    NLOC, NPAD, G = cfg["NLOC"], cfg["NPAD"], cfg["G"]
    NL_T = NPAD // 128
    CH = G * 128
    EPAD = CH * NL_T
    lo = core * NLOC
    sel = (dst_g >= lo) & (dst_g < lo + NLOC)
    es = src_g[sel]
    ed = dst_g[sel] - lo
    order = np.argsort(ed, kind="stable")
    es, ed = es[order], ed[order]

    src_idx = np.zeros(EPAD, dtype=np.int16)
    bt = np.zeros((NL_T, CH, 128), dtype=np.float32)
    btT = np.zeros((NL_T, 128, CH), dtype=np.float32)
    remap = lambda gidx: (gidx // NLOC) * NPAD + (gidx % NLOC)
    tile_of = ed // 128
    starts = np.searchsorted(tile_of, np.arange(NL_T), side="left")
    ends = np.searchsorted(tile_of, np.arange(NL_T), side="right")
    for t in range(NL_T):
        p0 = t * CH
        # group 0: self loops of local nodes t*128 .. t*128+127
        nids = t * 128 + np.arange(128)
        valid = nids < NLOC
        src_idx[p0:p0 + 128] = np.where(valid, remap(lo + nids), 0)
        vd = np.arange(128)[valid]
        bt[t, vd, vd] = 1.0
        btT[t, vd, vd] = 1.0
        # groups 1..: random edges of this tile
        a, b = starts[t], ends[t]
        cnt = b - a
        assert 128 + cnt <= CH, f"dst tile {t}: {cnt} edges > {CH - 128}"
        src_idx[p0 + 128:p0 + 128 + cnt] = remap(es[a:b]).astype(np.int16)
        dl = (ed[a:b] - t * 128).astype(np.int64)
        bt[t, 128 + np.arange(cnt), dl] = 1.0
        btT[t, dl, 128 + np.arange(cnt)] = 1.0
        if t < 2:
            src_idx[p0 + 128 + cnt:p0 + CH] = 0
    return src_idx, bt, btT


def wrap16(idx):
    """[EPAD] token array -> [128, EPAD/16] wrapped+replicated layout."""
    w = idx.reshape(-1, 16).T  # [16, EPAD/16]
    return np.ascontiguousarray(np.tile(w, (8, 1)))


def _bt_to_dram(bt, G):
    """bt [NL_T, CH, 128] (token-major) -> DRAM [NL_T, 128, CH] so that the
    SBUF tile [128, G, 128] slice [:, g, :] has token g*128+p at partition
    p: DRAM[k, p, g*128 + d] = bt[k, g*128 + p, d]."""
    NL_T, CH, _ = bt.shape
    out = bt.reshape(NL_T, G, 128, 128).transpose(0, 2, 1, 3)
    return np.ascontiguousarray(
        out.reshape(NL_T, 128, CH)).astype(ml_dtypes.bfloat16)


def host_prepare(cfg, x, pos, edge_index, batch,
                 W1, a_src1, a_dst1, b1, W2, a_src2, a_dst2, b2):
    PC, NG, NLOC, NPAD, H, C, HC, FIN, B = (
        cfg["PC"], cfg["NG"], cfg["NLOC"], cfg["NPAD"], cfg["H"], cfg["C"],
        cfg["HC"], cfg["FIN"], cfg["B"])
    G = cfg["G"]
    NL_T = NPAD // 128
    ACT_W = HC + 2 * H
    NTBL = PC * NPAD
    bf = ml_dtypes.bfloat16

    x_in = np.concatenate([pos, x], axis=1).astype(np.float32)  # [NG, FIN]
    src = np.asarray(edge_index[0])
    dst = np.asarray(edge_index[1])

    xpad = np.zeros((NTBL, FIN + 1), np.float32)
    xpad[:, FIN] = 1.0
    for c in range(PC):
        xpad[c * NPAD:c * NPAD + NLOC, 0:FIN] = x_in[c * NLOC:(c + 1) * NLOC]
    xt = np.ascontiguousarray(xpad.T).astype(bf)

    def augment(W, a_s, a_d, b):
        wad = np.einsum("fhc,hc->fh", W.reshape(W.shape[0], H, C), a_d)
        was = np.einsum("fhc,hc->fh", W.reshape(W.shape[0], H, C), a_s)
        waug = np.concatenate([W, was, wad], axis=1).astype(np.float32)
        cs = np.einsum("hc,hc->h", b.reshape(H, C), a_s)
        cd = np.einsum("hc,hc->h", b.reshape(H, C), a_d)
        brow = np.concatenate([b, cs, cd]).astype(np.float32)
        return waug, brow

    w1aug, b1row = augment(W1, a_src1, a_dst1, b1)
    w2aug, b2row = augment(W2, a_src2, a_dst2, b2)
    w1f = np.concatenate([w1aug, b1row[None, :]], axis=0).astype(bf)
    w2k = np.ascontiguousarray(
        w2aug.reshape(2, 128, ACT_W).transpose(1, 0, 2)).astype(bf)
    b2rv = b2row[None, :].astype(bf)
    ident = np.eye(128, dtype=np.float32)
    ones1 = np.ones((1, 128), dtype=bf)

    cnt = np.bincount(np.asarray(batch).astype(np.int64), minlength=B)
    in_maps = []
    for c in range(PC):
        si, bt, btT = pack_edges(cfg, src, dst, c)
        mp = np.zeros((NPAD, B), np.float32)
        gb = np.asarray(batch)[c * NLOC:(c + 1) * NLOC].astype(np.int64)
        mp[np.arange(NLOC), gb] = 1.0 / np.maximum(cnt[gb], 1.0)
        mpool = np.ascontiguousarray(
            mp.reshape(NL_T, 128, B).transpose(1, 0, 2)).astype(bf)
        in_maps.append(dict(
            xt=xt, w1=w1f, w2=w2k, b2r=b2rv, ones1=ones1, ident=ident,
            srcw=wrap16(si),
            btd=_bt_to_dram(bt, G),
            btTd=np.ascontiguousarray(btT).astype(bf),
            mpool=mpool,
        ))
    return in_maps


def host_tail(pooled_parts, lw1, lb1, lw2, lb2):
    pooled = np.sum(np.stack(pooled_parts), axis=0)
    y = np.maximum(pooled @ lw1 + lb1, 0.0)
    y = np.maximum(y @ lw2 + lb2, 0.0)
    return y.astype(np.float32)


# ----------------------------------------------------------------------------
# entry point
# ----------------------------------------------------------------------------
_CACHE = {}


def kernel(**inputs):
    from concourse.bass_utils import run_bass_kernel_spmd

    cfg = full_cfg()
    inp = {k: np.asarray(v) for k, v in inputs.items()}
    in_maps = host_prepare(
        cfg, inp["x"], inp["pos"], inp["edge_index"], inp["batch"],
        inp["W1"], inp["a_src1"], inp["a_dst1"], inp["b1"],
        inp["W2"], inp["a_src2"], inp["a_dst2"], inp["b2"])
    if "nc" not in _CACHE:
        _CACHE["nc"] = build_program(cfg)
    nc = _CACHE["nc"]
    res = run_bass_kernel_spmd(nc, in_maps, list(range(cfg["PC"])))
    parts = [res.results[c]["pooled"] for c in range(cfg["PC"])]
    return host_tail(parts, inp["lw1"], inp["lb1"], inp["lw2"], inp["lb2"])


# revision 13
# speedup vs baseline: 2.3240x; 1.1461x over previous
"""Trainium2 Bass kernel for nn_MultiGat (2-layer GAT + mean-pool + MLP).

Strategy (8 NeuronCores, SPMD single program), v2:
  - Nodes sharded 2500/core (padded 2560).  Each core owns the edges whose
    destination lands in its range, grouped per 128-node destination tile
    (chunk), padded to G groups of 128 edge tokens per chunk.
  - Table row per node (bf16, ROW=384 cols = 768 B): [h+b (256) | a_src (4)
    | a_dst (4) | pad].  Layer-1 table replicated per core; layer-2 table
    sharded + AllGather (as in the reference sharding hint).
  - Per chunk ONE dma_gather fetches the full src row per edge token
    (768 B, one descriptor per token).  Padding tokens use trailing -1
    indices, which the SWDGE Q7 kernel truncates before descriptor
    generation (chunks 0/1 of each layer pad with row 0 instead, because
    their SBUF buffers hold uninitialized bits on first use).
  - Token 0..127 of each chunk are the 128 destination nodes' self-loop
    edges in destination order, so the gathered group-0 rows double as the
    per-destination a_dst table ([128, H] aligned by partition) -- no
    second gather and no per-core dynamic addressing.
  - The edge->dst one-hot (bt) and its transpose (btT) are STATIC graph
    structure: host-precomputed bf16 DRAM tensors streamed by regular DMA
    (no Q7 descriptor cost, no per-group DVE is_equal builds).  btT gives
    per-edge a_dst via tiny matmuls adp_g = btT_g^T @ ad_tile; bt
    aggregates messages acc += bt_g^T @ [p*h | p] in PSUM (duplicate
    destinations accumulate natively).
  - Softmax uses exp without max-subtraction (values are O(1); the
    per-node normalizer cancels), with the reference's +1e-16 in the
    denominator.  leaky_relu and exp run on the Scalar engine.
  - Biases are folded: b is added to h during the table build via a
    ones-row in the stationary operand (softmax weights sum to 1), and the
    alpha contributions of b are host-folded into that bias row.
  - Mean-pool partials per core via a (1/cnt) matmul; host sums the 8
    partials and runs the tiny 256->128->10 MLP in numpy.
"""

import sys

sys.path.insert(0, "/opt/trn_rl_repo")

import numpy as np
import ml_dtypes


# ----------------------------------------------------------------------------
# configuration
# ----------------------------------------------------------------------------
def full_cfg():
    return dict(
        PC=8,          # cores
        NG=20000,      # global nodes
        NLOC=2500,     # nodes per core
        NPAD=2560,     # padded nodes per core (multiple of 128)
        H=4, C=64, HC=256,
        ROW=384,       # table row width (bf16): h(256) as(4) ad(4) pad(120)
        G=19,          # edge groups (of 128) per destination tile
        B=32,          # graphs
        FIN=64,        # input features (pos 2 + x 62)
    )


# ----------------------------------------------------------------------------
# device program
# ----------------------------------------------------------------------------

def _patch_tile_swdge_lane_by_queue():
    """Pin each Pool-engine DMA instruction's DMASW sem lane to its SWDGE
    queue_num (Tile's default round-robin mixes queues on one sem lane,
    which the scheduler rejects when num_swdge_queues > 1)."""
    import concourse.tile_sem_assignment as tsa
    if getattr(tsa, "_lane_by_queue_patched", False):
        return
    tsa._lane_by_queue_patched = True
    import concourse.mybir as mybir
    import concourse.bass_isa as bass_isa

    orig = tsa.TileClockTick._assign_tick

    def _assign_tick(self, inst):
        from concourse.tile_scheduler import DMAInst
        if (
            isinstance(inst, DMAInst)
            and not isinstance(inst, bass_isa.UserSyncedRemoteDMADescs)
            and inst.engine == mybir.EngineType.Pool
        ):
            q = int(getattr(inst, "queue_num", 0) or 0)
            self.next_sw_dma_idx = q
        return orig(self, inst)

    tsa.TileClockTick._assign_tick = _assign_tick


def build_program(cfg, reps=1):
    import concourse.mybir as mybir
    import concourse.bacc as bacc
    import concourse.tile as tile

    f32 = mybir.dt.float32
    bf16 = mybir.dt.bfloat16
    i16 = mybir.dt.int16
    AF = mybir.ActivationFunctionType

    PC, NPAD, ROW, HC, H, C, G = (
        cfg["PC"], cfg["NPAD"], cfg["ROW"], cfg["HC"], cfg["H"], cfg["C"],
        cfg["G"])
    B, FIN = cfg["B"], cfg["FIN"]
    NTBL = PC * NPAD           # table rows (global, padded)
    NT_T = NTBL // 128         # node tiles for table build
    NL_T = NPAD // 128         # local node tiles (= edge chunks per layer)
    CH = G * 128               # tokens per chunk (one dst tile)
    EPAD = CH * NL_T           # padded edge tokens per core
    ICOLS = EPAD // 16
    NAUG = HC + H              # aggregated row: [msg(256) | p(4)]
    ACT_W = HC + 2 * H         # active row columns: h | a_src | a_dst

    _patch_tile_swdge_lane_by_queue()
    NQ = cfg.get("NQ", 4)
    nc = bacc.Bacc(None, target_bir_lowering=False, debug=True,
                   num_swdge_queues=NQ)

    # ---- I/O
    # xt carries an extra all-ones row (row FIN) so the table matmul adds
    # w1's bias row directly.
    xt = nc.declare_dram_parameter("xt", [FIN + 1, NTBL], bf16, isOutput=False)
    w1 = nc.declare_dram_parameter("w1", [FIN + 1, ACT_W], bf16,
                                   isOutput=False)
    w2 = nc.declare_dram_parameter("w2", [128, 2, ACT_W], bf16,
                                   isOutput=False)
    b2r = nc.declare_dram_parameter("b2r", [1, ACT_W], bf16, isOutput=False)
    ones1 = nc.declare_dram_parameter("ones1", [1, 128], bf16, isOutput=False)
    ident = nc.declare_dram_parameter("ident", [128, 128], f32,
                                      isOutput=False)
    srcw = nc.declare_dram_parameter("srcw", [128, ICOLS], i16, isOutput=False)
    btd = nc.declare_dram_parameter("btd", [NL_T, 128, CH], bf16,
                                    isOutput=False)
    btTd = nc.declare_dram_parameter("btTd", [NL_T, 128, CH], bf16,
                                     isOutput=False)
    mpool = nc.declare_dram_parameter("mpool", [128, NL_T, B], bf16,
                                      isOutput=False)
    pooled = nc.declare_dram_parameter("pooled", [B, HC], f32, isOutput=True)

    # ---- internal DRAM
    T1 = nc.dram_tensor("T1", [NTBL, ROW], bf16)
    T2s = nc.dram_tensor("T2s", [NPAD, ROW], bf16)
    T2 = nc.dram_tensor("T2", [NTBL, ROW], bf16, addr_space="Shared")

    with tile.TileContext(nc) as tc:
        with tc.tile_pool(name="persist", bufs=1) as pp:
            si = pp.tile([128, ICOLS], i16)
            on1 = pp.tile([1, 128], bf16)
            ids = pp.tile([128, 128], f32)
            nc.sync.dma_start(si[:], srcw[:])
            nc.sync.dma_start(on1[:], ones1[:])
            nc.sync.dma_start(ids[:], ident[:])

            for _rep in range(reps):
                # ------------- phase 0: build T1 (replicated) -------------
                with (
                    tc.tile_pool(name="p0", bufs=3) as p0,
                    tc.tile_pool(name="p0w", bufs=1) as p0w,
                    tc.tile_pool(name="ps0", bufs=4, space="PSUM") as ps0,
                ):
                    xts = p0w.tile([FIN + 1, NTBL], bf16)
                    nc.sync.dma_start(xts[:], xt[:])
                    w1s = p0w.tile([FIN + 1, ACT_W], bf16)
                    nc.sync.dma_start(w1s[:], w1[:])
                    for j in range(0, NT_T, 2):
                        j0 = j * 128
                        ps = ps0.tile([128, 2, 512], f32)
                        for i in range(2):
                            nc.tensor.matmul(
                                ps[:, i, 0:ACT_W],
                                xts[:, j0 + i * 128:j0 + (i + 1) * 128],
                                w1s[:], start=True, stop=True)
                        ts = p0.tile([128, 2, ACT_W], bf16)
                        if (j // 2) % 2 == 0:
                            nc.scalar.activation(ts[:], ps[:, :, 0:ACT_W],
                                                 AF.Copy)
                        else:
                            nc.vector.tensor_copy(ts[:], ps[:, :, 0:ACT_W])
                        nc.sync.dma_start(
                            T1[j0:j0 + 256, 0:ACT_W].rearrange(
                                "(t p) w -> p t w", p=128), ts[:])

                # ------------- edge phase: one dst tile per chunk ----------
                # consume(j, o) receives the normalized output tile
                # o [128, HC] (bf16) for local node tile j.
                def edge_phase(T, tag, consume):
                    with (
                        tc.tile_pool(name=f"e{tag}", bufs=3) as ep,
                        tc.tile_pool(name=f"eb{tag}", bufs=3) as ebp,
                        tc.tile_pool(name=f"es{tag}", bufs=4) as esp,
                        tc.tile_pool(name=f"eps{tag}", bufs=2,
                                     space="PSUM") as epsp,
                        tc.tile_pool(name=f"aps{tag}", bufs=2,
                                     space="PSUM") as apsp,
                    ):
                        for k in range(NL_T):
                            cols = slice(k * (CH // 16), (k + 1) * (CH // 16))
                            bts = ebp.tile([128, G, 128], bf16, tag="bt")
                            nc.sync.dma_start(
                                bts[:],
                                btd[k].rearrange("p (g d) -> p g d", g=G))
                            btTs = ebp.tile([128, G, 128], bf16, tag="btT")
                            nc.sync.dma_start(
                                btTs[:],
                                btTd[k].rearrange("p (g e) -> p g e", g=G))
                            g1 = ep.tile([128, G, ROW], bf16, tag="g1")
                            nc.gpsimd.dma_gather(
                                g1[:], T[:, :], si[:, cols], CH, CH, ROW,
                                elem_step=ROW, single_packet=False,
                                queue_num=k % NQ)
                            # per-edge a_dst via one-hot-transpose matmuls;
                            # the ad table is the gathered self-loop rows.
                            adt = g1[:, 0, HC + H:HC + 2 * H]
                            adp = apsp.tile([128, G, H], f32, tag="adp")
                            for g in range(G):
                                nc.tensor.matmul(
                                    adp[:, g, :], btTs[:, g, :], adt,
                                    start=True, stop=True)
                            se = esp.tile([128, G, H], f32, tag="se")
                            nc.vector.tensor_add(
                                se[:], g1[:, :, HC:HC + H], adp[:])
                            lr = esp.tile([128, G, H], f32, tag="lr")
                            nc.vector.tensor_scalar_mul(lr[:], se[:], 0.2)
                            lr2 = esp.tile([128, G, H], f32, tag="lr2")
                            nc.vector.tensor_max(lr2[:], se[:], lr[:])
                            mp = ep.tile([128, G, NAUG], bf16, tag="mp")
                            pv = mp[:, :, HC:HC + H]
                            nc.scalar.activation(pv, lr2[:], AF.Exp)
                            pb = pv.unsqueeze(3).broadcast_to([128, G, H, C])
                            nc.vector.tensor_mul(
                                mp[:, :, 0:HC].rearrange(
                                    "p m (h c) -> p m h c", c=C),
                                g1[:, :, 0:HC].rearrange(
                                    "p m (h c) -> p m h c", c=C),
                                pb)
                            # aggregate via one-hot matmuls
                            acc = epsp.tile([128, NAUG], f32, tag="acc")
                            for g in range(G):
                                nc.tensor.matmul(
                                    acc[:], bts[:, g, :], mp[:, g, :],
                                    start=(g == 0), stop=(g == G - 1))
                            # normalize: o = num / (den + 1e-16)
                            nc.vector.tensor_scalar_add(
                                acc[:, HC:HC + H], acc[:, HC:HC + H], 1e-16)
                            rd = esp.tile([128, H], f32, tag="rd")
                            nc.vector.reciprocal(rd[:], acc[:, HC:HC + H])
                            o = esp.tile([128, HC], f32, tag="o")
                            rb = rd[:].unsqueeze(2).broadcast_to([128, H, C])
                            nc.vector.tensor_mul(
                                o[:].rearrange("p (h c) -> p h c", c=C),
                                acc[:, 0:HC].rearrange(
                                    "p (h c) -> p h c", c=C),
                                rb)
                            consume(k, o)

                # ------------- layer 1 + transpose into o1T -------------
                with (
                    tc.tile_pool(name="p2w", bufs=1) as p2w,
                    tc.tile_pool(name="p2", bufs=3) as p2,
                    tc.tile_pool(name="pst", bufs=2, space="PSUM") as pst,
                    tc.tile_pool(name="ps2", bufs=2, space="PSUM") as ps2,
                ):
                    o1T = p2w.tile([128, 2, NPAD], bf16)
                    w2s = p2w.tile([128, 2, ACT_W], bf16)
                    nc.sync.dma_start(w2s[:], w2[:])
                    b2s = p2w.tile([1, ACT_W], bf16)
                    nc.sync.dma_start(b2s[:], b2r[:])

                    def consume1(j, o):
                        j0 = j * 128
                        for kk in range(2):
                            pt = pst.tile([128, 128], f32, tag="pt")
                            nc.tensor.transpose(
                                pt[:], o[:, kk * 128:(kk + 1) * 128], ids[:])
                            nc.vector.tensor_copy(
                                o1T[:, kk, j0:j0 + 128], pt[:])
                        ps = ps2.tile([128, ACT_W], f32, tag="mm")
                        nc.tensor.matmul(ps[:], o1T[:, 0, j0:j0 + 128],
                                         w2s[:, 0, :], start=True, stop=False)
                        nc.tensor.matmul(ps[:], o1T[:, 1, j0:j0 + 128],
                                         w2s[:, 1, :], start=False, stop=False)
                        nc.tensor.matmul(ps[:], on1[:], b2s[:],
                                         start=False, stop=True)
                        ts = p2.tile([128, ACT_W], bf16, tag="t2row")
                        nc.scalar.activation(ts[:], ps[:], AF.Copy)
                        nc.sync.dma_start(T2s[j0:j0 + 128, 0:ACT_W], ts[:])

                    edge_phase(T1, "1", consume1)

                    nc.gpsimd.collective_compute(
                        "AllGather",
                        mybir.AluOpType.bypass,
                        replica_groups=[list(range(PC))],
                        ins=[T2s[:].bitcast(f32)],
                        outs=[T2[:].bitcast(f32)],
                    )

                # ------------- layer 2 + pooling -------------
                with (
                    tc.tile_pool(name="p4w", bufs=1) as p4w,
                    tc.tile_pool(name="ps4", bufs=2, space="PSUM") as ps4,
                ):
                    o2buf = p4w.tile([128, NL_T, HC], bf16)

                    def consume2(j, o):
                        nc.vector.tensor_copy(o2buf[:, j, :], o[:])

                    edge_phase(T2, "2", consume2)

                    mps = p4w.tile([128, NL_T, B], bf16)
                    nc.sync.dma_start(mps[:], mpool[:])
                    acc = ps4.tile([B, HC], f32)
                    for j in range(NL_T):
                        nc.tensor.matmul(acc[:], mps[:, j, :], o2buf[:, j, :],
                                         start=(j == 0), stop=(j == NL_T - 1))
                    po = p4w.tile([B, HC], f32)
                    nc.vector.tensor_copy(po[:], acc[:])
                    nc.sync.dma_start(pooled[:], po[:])

        _, _snap = tc.schedule_and_allocate()
        nc.predicted_ns = _snap.time if _snap is not None else None

    nc.compile()
    return nc


# ----------------------------------------------------------------------------
# host-side preparation
# ----------------------------------------------------------------------------
def pack_edges(cfg, src_g, dst_g, core):
    """Build this core's edge-token stream.  Per 128-dst tile (chunk):
    tokens 0..127 are the tile's self-loop edges in destination order
    (token d = self loop of local dst d, so the gathered group-0 rows
    serve as the per-destination a_dst table); tokens 128.. are the core's
    incident random edges sorted by destination.  Padding uses -1 (Q7
    drops trailing negatives) except chunks 0/1 and non-trailing slots,
    which point at row 0.  Returns (src_idx [EPAD] int16,
    bt [NL_T, CH, 128], btT [NL_T, 128, CH]) with bt[k, t, d] = 1 iff
    token t of chunk k targets local dst d."""
    NLOC, NPAD, G = cfg["NLOC"], cfg["NPAD"], cfg["G"]
    NL_T = NPAD // 128
    CH = G * 128
    EPAD = CH * NL_T
    lo = core * NLOC
    sel = (dst_g >= lo) & (dst_g < lo + NLOC)
    es = src_g[sel]
    ed = dst_g[sel] - lo
    order = np.argsort(ed, kind="stable")
    es, ed = es[order], ed[order]

    src_idx = np.zeros(EPAD, dtype=np.int16)
    bt = np.zeros((NL_T, CH, 128), dtype=np.float32)
    btT = np.zeros((NL_T, 128, CH), dtype=np.float32)
    remap = lambda gidx: (gidx // NLOC) * NPAD + (gidx % NLOC)
    tile_of = ed // 128
    starts = np.searchsorted(tile_of, np.arange(NL_T), side="left")
    ends = np.searchsorted(tile_of, np.arange(NL_T), side="right")
    for t in range(NL_T):
        p0 = t * CH
        # group 0: self loops of local nodes t*128 .. t*128+127
        nids = t * 128 + np.arange(128)
        valid = nids < NLOC
        src_idx[p0:p0 + 128] = np.where(valid, remap(lo + nids), 0)
        vd = np.arange(128)[valid]
        bt[t, vd, vd] = 1.0
        btT[t, vd, vd] = 1.0
        # groups 1..: random edges of this tile
        a, b = starts[t], ends[t]
        cnt = b - a
        assert 128 + cnt <= CH, f"dst tile {t}: {cnt} edges > {CH - 128}"
        src_idx[p0 + 128:p0 + 128 + cnt] = remap(es[a:b]).astype(np.int16)
        dl = (ed[a:b] - t * 128).astype(np.int64)
        bt[t, 128 + np.arange(cnt), dl] = 1.0
        btT[t, dl, 128 + np.arange(cnt)] = 1.0
        if t < 2:
            src_idx[p0 + 128 + cnt:p0 + CH] = 0
    return src_idx, bt, btT


def wrap16(idx):
    """[EPAD] token array -> [128, EPAD/16] wrapped+replicated layout."""
    w = idx.reshape(-1, 16).T  # [16, EPAD/16]
    return np.ascontiguousarray(np.tile(w, (8, 1)))


def _bt_to_dram(bt, G):
    """bt [NL_T, CH, 128] (token-major) -> DRAM [NL_T, 128, CH] so that the
    SBUF tile [128, G, 128] slice [:, g, :] has token g*128+p at partition
    p: DRAM[k, p, g*128 + d] = bt[k, g*128 + p, d]."""
    NL_T, CH, _ = bt.shape
    out = bt.reshape(NL_T, G, 128, 128).transpose(0, 2, 1, 3)
    return np.ascontiguousarray(
        out.reshape(NL_T, 128, CH)).astype(ml_dtypes.bfloat16)


def host_prepare(cfg, x, pos, edge_index, batch,
                 W1, a_src1, a_dst1, b1, W2, a_src2, a_dst2, b2):
    PC, NG, NLOC, NPAD, H, C, HC, FIN, B = (
        cfg["PC"], cfg["NG"], cfg["NLOC"], cfg["NPAD"], cfg["H"], cfg["C"],
        cfg["HC"], cfg["FIN"], cfg["B"])
    G = cfg["G"]
    NL_T = NPAD // 128
    ACT_W = HC + 2 * H
    NTBL = PC * NPAD
    bf = ml_dtypes.bfloat16

    x_in = np.concatenate([pos, x], axis=1).astype(np.float32)  # [NG, FIN]
    src = np.asarray(edge_index[0])
    dst = np.asarray(edge_index[1])

    xpad = np.zeros((NTBL, FIN + 1), np.float32)
    xpad[:, FIN] = 1.0
    for c in range(PC):
        xpad[c * NPAD:c * NPAD + NLOC, 0:FIN] = x_in[c * NLOC:(c + 1) * NLOC]
    xt = np.ascontiguousarray(xpad.T).astype(bf)

    def augment(W, a_s, a_d, b):
        wad = np.einsum("fhc,hc->fh", W.reshape(W.shape[0], H, C), a_d)
        was = np.einsum("fhc,hc->fh", W.reshape(W.shape[0], H, C), a_s)
        waug = np.concatenate([W, was, wad], axis=1).astype(np.float32)
        cs = np.einsum("hc,hc->h", b.reshape(H, C), a_s)
        cd = np.einsum("hc,hc->h", b.reshape(H, C), a_d)
        brow = np.concatenate([b, cs, cd]).astype(np.float32)
        return waug, brow

    w1aug, b1row = augment(W1, a_src1, a_dst1, b1)
    w2aug, b2row = augment(W2, a_src2, a_dst2, b2)
    w1f = np.concatenate([w1aug, b1row[None, :]], axis=0).astype(bf)
    w2k = np.ascontiguousarray(
        w2aug.reshape(2, 128, ACT_W).transpose(1, 0, 2)).astype(bf)
    b2rv = b2row[None, :].astype(bf)
    ident = np.eye(128, dtype=np.float32)
    ones1 = np.ones((1, 128), dtype=bf)

    cnt = np.bincount(np.asarray(batch).astype(np.int64), minlength=B)
    in_maps = []
    for c in range(PC):
        si, bt, btT = pack_edges(cfg, src, dst, c)
        mp = np.zeros((NPAD, B), np.float32)
        gb = np.asarray(batch)[c * NLOC:(c + 1) * NLOC].astype(np.int64)
        mp[np.arange(NLOC), gb] = 1.0 / np.maximum(cnt[gb], 1.0)
        mpool = np.ascontiguousarray(
            mp.reshape(NL_T, 128, B).transpose(1, 0, 2)).astype(bf)
        in_maps.append(dict(
            xt=xt, w1=w1f, w2=w2k, b2r=b2rv, ones1=ones1, ident=ident,
            srcw=wrap16(si),
            btd=_bt_to_dram(bt, G),
            btTd=np.ascontiguousarray(btT).astype(bf),
            mpool=mpool,
        ))
    return in_maps


def host_tail(pooled_parts, lw1, lb1, lw2, lb2):
    pooled = np.sum(np.stack(pooled_parts), axis=0)
    y = np.maximum(pooled @ lw1 + lb1, 0.0)
    y = np.maximum(y @ lw2 + lb2, 0.0)
    return y.astype(np.float32)


# ----------------------------------------------------------------------------
# entry point
# ----------------------------------------------------------------------------
_CACHE = {}


def kernel(**inputs):
    from concourse.bass_utils import run_bass_kernel_spmd

    cfg = full_cfg()
    inp = {k: np.asarray(v) for k, v in inputs.items()}
    in_maps = host_prepare(
        cfg, inp["x"], inp["pos"], inp["edge_index"], inp["batch"],
        inp["W1"], inp["a_src1"], inp["a_dst1"], inp["b1"],
        inp["W2"], inp["a_src2"], inp["a_dst2"], inp["b2"])
    if "nc" not in _CACHE:
        _CACHE["nc"] = build_program(cfg)
    nc = _CACHE["nc"]
    res = run_bass_kernel_spmd(nc, in_maps, list(range(cfg["PC"])))
    parts = [res.results[c]["pooled"] for c in range(cfg["PC"])]
    return host_tail(parts, inp["lw1"], inp["lb1"], inp["lw2"], inp["lb2"])


# revision 14
# speedup vs baseline: 2.7262x; 1.1731x over previous
"""Trainium2 Bass kernel for nn_MultiGat (2-layer GAT + mean-pool + MLP).

Strategy (8 NeuronCores, SPMD single program), v2:
  - Nodes sharded 2500/core (padded 2560).  Each core owns the edges whose
    destination lands in its range, grouped per 128-node destination tile
    (chunk), padded to G groups of 128 edge tokens per chunk.
  - Table row per node (bf16, ROW=384 cols = 768 B): [h+b (256) | a_src (4)
    | a_dst (4) | pad].  Layer-1 table replicated per core; layer-2 table
    sharded + AllGather (as in the reference sharding hint).
  - Per chunk ONE dma_gather fetches the full src row per edge token
    (768 B, one descriptor per token).  Padding tokens use trailing -1
    indices, which the SWDGE Q7 kernel truncates before descriptor
    generation (chunks 0/1 of each layer pad with row 0 instead, because
    their SBUF buffers hold uninitialized bits on first use).
  - Token 0..127 of each chunk are the 128 destination nodes' self-loop
    edges in destination order, so the gathered group-0 rows double as the
    per-destination a_dst table ([128, H] aligned by partition) -- no
    second gather and no per-core dynamic addressing.
  - The edge->dst one-hot (bt) and its transpose (btT) are STATIC graph
    structure: host-precomputed bf16 DRAM tensors streamed by regular DMA
    (no Q7 descriptor cost, no per-group DVE is_equal builds).  btT gives
    per-edge a_dst via tiny matmuls adp_g = btT_g^T @ ad_tile; bt
    aggregates messages acc += bt_g^T @ [p*h | p] in PSUM (duplicate
    destinations accumulate natively).
  - Softmax uses exp without max-subtraction (values are O(1); the
    per-node normalizer cancels), with the reference's +1e-16 in the
    denominator.  leaky_relu and exp run on the Scalar engine.
  - Biases are folded: b is added to h during the table build via a
    ones-row in the stationary operand (softmax weights sum to 1), and the
    alpha contributions of b are host-folded into that bias row.
  - Mean-pool partials per core via a (1/cnt) matmul; host sums the 8
    partials and runs the tiny 256->128->10 MLP in numpy.
"""

import sys

sys.path.insert(0, "/opt/trn_rl_repo")

import numpy as np
import ml_dtypes


# ----------------------------------------------------------------------------
# configuration
# ----------------------------------------------------------------------------
def full_cfg():
    return dict(
        PC=8,          # cores
        NG=20000,      # global nodes
        NLOC=2500,     # nodes per core
        NPAD=2560,     # padded nodes per core (multiple of 128)
        H=4, C=64, HC=256,
        ROW=384,       # table row width (bf16): h(256) as(4) ad(4) pad(120)
        G=17,          # edge groups (of 128) per destination tile
        B=32,          # graphs
        FIN=64,        # input features (pos 2 + x 62)
    )


# ----------------------------------------------------------------------------
# device program
# ----------------------------------------------------------------------------

def _patch_tile_swdge_lane_by_queue():
    """Pin each Pool-engine DMA instruction's DMASW sem lane to its SWDGE
    queue_num (Tile's default round-robin mixes queues on one sem lane,
    which the scheduler rejects when num_swdge_queues > 1)."""
    import concourse.tile_sem_assignment as tsa
    if getattr(tsa, "_lane_by_queue_patched", False):
        return
    tsa._lane_by_queue_patched = True
    import concourse.mybir as mybir
    import concourse.bass_isa as bass_isa

    orig = tsa.TileClockTick._assign_tick

    def _assign_tick(self, inst):
        from concourse.tile_scheduler import DMAInst
        if (
            isinstance(inst, DMAInst)
            and not isinstance(inst, bass_isa.UserSyncedRemoteDMADescs)
            and inst.engine == mybir.EngineType.Pool
        ):
            q = int(getattr(inst, "queue_num", 0) or 0)
            self.next_sw_dma_idx = q
        return orig(self, inst)

    tsa.TileClockTick._assign_tick = _assign_tick


def build_program(cfg, reps=1):
    import concourse.mybir as mybir
    import concourse.bacc as bacc
    import concourse.tile as tile

    f32 = mybir.dt.float32
    bf16 = mybir.dt.bfloat16
    i16 = mybir.dt.int16
    AF = mybir.ActivationFunctionType

    PC, NPAD, ROW, HC, H, C, G = (
        cfg["PC"], cfg["NPAD"], cfg["ROW"], cfg["HC"], cfg["H"], cfg["C"],
        cfg["G"])
    B, FIN = cfg["B"], cfg["FIN"]
    NTBL = PC * NPAD           # table rows (global, padded)
    NT_T = NTBL // 128         # node tiles for table build
    NL_T = NPAD // 128         # local node tiles (= edge chunks per layer)
    CH = G * 128               # tokens per chunk (one dst tile)
    EPAD = CH * NL_T           # padded edge tokens per core
    ICOLS = EPAD // 16
    NAUG = HC + H              # aggregated row: [msg(256) | p(4)]
    ACT_W = HC + 2 * H         # active row columns: h | a_src | a_dst

    _patch_tile_swdge_lane_by_queue()
    NQ = cfg.get("NQ", 4)
    nc = bacc.Bacc(None, target_bir_lowering=False, debug=True,
                   num_swdge_queues=NQ)

    # ---- I/O
    # xt carries an extra all-ones row (row FIN) so the table matmul adds
    # w1's bias row directly.
    xt = nc.declare_dram_parameter("xt", [FIN + 1, NTBL], bf16, isOutput=False)
    w1 = nc.declare_dram_parameter("w1", [FIN + 1, ACT_W], bf16,
                                   isOutput=False)
    w2 = nc.declare_dram_parameter("w2", [128, 2, ACT_W], bf16,
                                   isOutput=False)
    b2r = nc.declare_dram_parameter("b2r", [1, ACT_W], bf16, isOutput=False)
    ones1 = nc.declare_dram_parameter("ones1", [1, 128], bf16, isOutput=False)
    ident = nc.declare_dram_parameter("ident", [128, 128], f32,
                                      isOutput=False)
    srcw = nc.declare_dram_parameter("srcw", [128, ICOLS], i16, isOutput=False)
    btd = nc.declare_dram_parameter("btd", [NL_T, 128, CH], bf16,
                                    isOutput=False)
    btTd = nc.declare_dram_parameter("btTd", [NL_T, 128, CH], bf16,
                                     isOutput=False)
    mpool = nc.declare_dram_parameter("mpool", [128, NL_T, B], bf16,
                                      isOutput=False)
    pooled = nc.declare_dram_parameter("pooled", [B, HC], f32, isOutput=True)

    # ---- internal DRAM
    T1 = nc.dram_tensor("T1", [NTBL, ROW], bf16)
    T2s = nc.dram_tensor("T2s", [NPAD, ROW], bf16)
    T2 = nc.dram_tensor("T2", [NTBL, ROW], bf16, addr_space="Shared")

    with tile.TileContext(nc) as tc:
        with tc.tile_pool(name="persist", bufs=1) as pp:
            si = pp.tile([128, ICOLS], i16)
            on1 = pp.tile([1, 128], bf16)
            ids = pp.tile([128, 128], f32)
            nc.sync.dma_start(si[:], srcw[:])
            nc.sync.dma_start(on1[:], ones1[:])
            nc.sync.dma_start(ids[:], ident[:])

            for _rep in range(reps):
                # ------------- phase 0: build T1 (replicated) -------------
                with (
                    tc.tile_pool(name="p0", bufs=3) as p0,
                    tc.tile_pool(name="p0w", bufs=1) as p0w,
                    tc.tile_pool(name="ps0", bufs=4, space="PSUM") as ps0,
                ):
                    xts = p0w.tile([FIN + 1, NTBL], bf16)
                    nc.sync.dma_start(xts[:], xt[:])
                    w1s = p0w.tile([FIN + 1, ACT_W], bf16)
                    nc.sync.dma_start(w1s[:], w1[:])
                    for j in range(0, NT_T, 2):
                        j0 = j * 128
                        ps = ps0.tile([128, 2, 512], f32)
                        for i in range(2):
                            nc.tensor.matmul(
                                ps[:, i, 0:ACT_W],
                                xts[:, j0 + i * 128:j0 + (i + 1) * 128],
                                w1s[:], start=True, stop=True)
                        ts = p0.tile([128, 2, ACT_W], bf16)
                        if (j // 2) % 2 == 0:
                            nc.scalar.activation(ts[:], ps[:, :, 0:ACT_W],
                                                 AF.Copy)
                        else:
                            nc.vector.tensor_copy(ts[:], ps[:, :, 0:ACT_W])
                        nc.sync.dma_start(
                            T1[j0:j0 + 256, 0:ACT_W].rearrange(
                                "(t p) w -> p t w", p=128), ts[:])

                # ------------- edge phase: one dst tile per chunk ----------
                # consume(j, o) receives the normalized output tile
                # o [128, HC] (bf16) for local node tile j.
                def edge_phase(T, tag, consume):
                    with (
                        tc.tile_pool(name=f"e{tag}", bufs=3) as ep,
                        tc.tile_pool(name=f"eb{tag}", bufs=3) as ebp,
                        tc.tile_pool(name=f"es{tag}", bufs=4) as esp,
                        tc.tile_pool(name=f"eps{tag}", bufs=2,
                                     space="PSUM") as epsp,
                        tc.tile_pool(name=f"aps{tag}", bufs=2,
                                     space="PSUM") as apsp,
                    ):
                        for k in range(NL_T):
                            cols = slice(k * (CH // 16), (k + 1) * (CH // 16))
                            bts = ebp.tile([128, G, 128], bf16, tag="bt")
                            nc.sync.dma_start(
                                bts[:],
                                btd[k].rearrange("p (g d) -> p g d", g=G))
                            btTs = ebp.tile([128, G, 128], bf16, tag="btT")
                            nc.sync.dma_start(
                                btTs[:],
                                btTd[k].rearrange("p (g e) -> p g e", g=G))
                            g1 = ep.tile([128, G, ROW], bf16, tag="g1")
                            nc.gpsimd.dma_gather(
                                g1[:], T[:, :], si[:, cols], CH, CH, ROW,
                                elem_step=ROW, single_packet=False,
                                queue_num=k % NQ)
                            # per-edge a_dst via one-hot-transpose matmuls;
                            # the ad table is the gathered self-loop rows.
                            adt = g1[:, 0, HC + H:HC + 2 * H]
                            adp = apsp.tile([128, G, H], f32, tag="adp")
                            for g in range(G):
                                nc.tensor.matmul(
                                    adp[:, g, :], btTs[:, g, :], adt,
                                    start=True, stop=True)
                            se = esp.tile([128, G, H], f32, tag="se")
                            nc.vector.tensor_add(
                                se[:], g1[:, :, HC:HC + H], adp[:])
                            lr = esp.tile([128, G, H], f32, tag="lr")
                            nc.vector.tensor_scalar_mul(lr[:], se[:], 0.2)
                            lr2 = esp.tile([128, G, H], f32, tag="lr2")
                            nc.vector.tensor_max(lr2[:], se[:], lr[:])
                            mp = ep.tile([128, G, NAUG], bf16, tag="mp")
                            pv = mp[:, :, HC:HC + H]
                            nc.scalar.activation(pv, lr2[:], AF.Exp)
                            pb = pv.unsqueeze(3).broadcast_to([128, G, H, C])
                            nc.vector.tensor_mul(
                                mp[:, :, 0:HC].rearrange(
                                    "p m (h c) -> p m h c", c=C),
                                g1[:, :, 0:HC].rearrange(
                                    "p m (h c) -> p m h c", c=C),
                                pb)
                            # aggregate via one-hot matmuls
                            acc = epsp.tile([128, NAUG], f32, tag="acc")
                            for g in range(G):
                                nc.tensor.matmul(
                                    acc[:], bts[:, g, :], mp[:, g, :],
                                    start=(g == 0), stop=(g == G - 1))
                            # normalize: o = num / (den + 1e-16)
                            nc.vector.tensor_scalar_add(
                                acc[:, HC:HC + H], acc[:, HC:HC + H], 1e-16)
                            rd = esp.tile([128, H], f32, tag="rd")
                            nc.vector.reciprocal(rd[:], acc[:, HC:HC + H])
                            o = esp.tile([128, HC], f32, tag="o")
                            rb = rd[:].unsqueeze(2).broadcast_to([128, H, C])
                            nc.vector.tensor_mul(
                                o[:].rearrange("p (h c) -> p h c", c=C),
                                acc[:, 0:HC].rearrange(
                                    "p (h c) -> p h c", c=C),
                                rb)
                            consume(k, o)

                # ------------- layer 1 + transpose into o1T -------------
                with (
                    tc.tile_pool(name="p2w", bufs=1) as p2w,
                    tc.tile_pool(name="p2", bufs=3) as p2,
                    tc.tile_pool(name="pst", bufs=2, space="PSUM") as pst,
                    tc.tile_pool(name="ps2", bufs=2, space="PSUM") as ps2,
                ):
                    o1T = p2w.tile([128, 2, NPAD], bf16)
                    w2s = p2w.tile([128, 2, ACT_W], bf16)
                    nc.sync.dma_start(w2s[:], w2[:])
                    b2s = p2w.tile([1, ACT_W], bf16)
                    nc.sync.dma_start(b2s[:], b2r[:])

                    def consume1(j, o):
                        j0 = j * 128
                        for kk in range(2):
                            pt = pst.tile([128, 128], f32, tag="pt")
                            nc.tensor.transpose(
                                pt[:], o[:, kk * 128:(kk + 1) * 128], ids[:])
                            nc.vector.tensor_copy(
                                o1T[:, kk, j0:j0 + 128], pt[:])
                        ps = ps2.tile([128, ACT_W], f32, tag="mm")
                        nc.tensor.matmul(ps[:], o1T[:, 0, j0:j0 + 128],
                                         w2s[:, 0, :], start=True, stop=False)
                        nc.tensor.matmul(ps[:], o1T[:, 1, j0:j0 + 128],
                                         w2s[:, 1, :], start=False, stop=False)
                        nc.tensor.matmul(ps[:], on1[:], b2s[:],
                                         start=False, stop=True)
                        ts = p2.tile([128, ACT_W], bf16, tag="t2row")
                        nc.scalar.activation(ts[:], ps[:], AF.Copy)
                        nc.sync.dma_start(T2s[j0:j0 + 128, 0:ACT_W], ts[:])

                    edge_phase(T1, "1", consume1)

                    nc.gpsimd.collective_compute(
                        "AllGather",
                        mybir.AluOpType.bypass,
                        replica_groups=[list(range(PC))],
                        ins=[T2s[:].bitcast(f32)],
                        outs=[T2[:].bitcast(f32)],
                    )

                # ------------- layer 2 + pooling -------------
                with (
                    tc.tile_pool(name="p4w", bufs=1) as p4w,
                    tc.tile_pool(name="ps4", bufs=2, space="PSUM") as ps4,
                ):
                    o2buf = p4w.tile([128, NL_T, HC], bf16)

                    def consume2(j, o):
                        nc.vector.tensor_copy(o2buf[:, j, :], o[:])

                    edge_phase(T2, "2", consume2)

                    mps = p4w.tile([128, NL_T, B], bf16)
                    nc.sync.dma_start(mps[:], mpool[:])
                    acc = ps4.tile([B, HC], f32)
                    for j in range(NL_T):
                        nc.tensor.matmul(acc[:], mps[:, j, :], o2buf[:, j, :],
                                         start=(j == 0), stop=(j == NL_T - 1))
                    po = p4w.tile([B, HC], f32)
                    nc.vector.tensor_copy(po[:], acc[:])
                    nc.sync.dma_start(pooled[:], po[:])

        _, _snap = tc.schedule_and_allocate()
        nc.predicted_ns = _snap.time if _snap is not None else None

    nc.compile()
    return nc


# ----------------------------------------------------------------------------
# host-side preparation
# ----------------------------------------------------------------------------
def balance_nodes(deg, n_tiles, cap=128):
    """LPT-balance nodes into n_tiles tiles of <= cap nodes so per-tile
    incident-edge counts are nearly equal.  Returns pos[n] = tile*128+slot."""
    import heapq
    order = np.argsort(-deg, kind="stable")
    counts = np.zeros(n_tiles, np.int64)
    heap = [(0, t) for t in range(n_tiles)]
    heapq.heapify(heap)
    pos = np.empty(len(deg), np.int64)
    spill = []
    for n in order:
        while True:
            load, t = heapq.heappop(heap)
            if counts[t] < cap:
                break
            spill.append((load, t))
        for e in spill:
            pass  # full tiles stay out of the heap
        spill.clear()
        pos[n] = t * 128 + counts[t]
        counts[t] += 1
        if counts[t] < cap:
            heapq.heappush(heap, (load + deg[n], t))
    return pos, counts


def pack_edges(cfg, src_pos, dst_pos, tile_counts, core):
    """Build this core's edge-token stream (positions are balanced table
    rows).  Per 128-dst tile (chunk): tokens 0..127 are the tile's
    self-loop edges in destination order (token d = self loop of local dst
    d, so the gathered group-0 rows double as the per-destination a_dst
    table; slots with no node point at row 0 and are masked out of the
    one-hots); tokens 128.. are the core's incident edges sorted by
    destination, padded with row 0.  Returns (src_idx [EPAD] int16,
    bt [NL_T, CH, 128], btT [NL_T, 128, CH]) with bt[k, t, d] = 1 iff
    token t of chunk k targets local dst d."""
    NPAD, G = cfg["NPAD"], cfg["G"]
    NL_T = NPAD // 128
    CH = G * 128
    EPAD = CH * NL_T
    lo = core * NPAD
    sel = (dst_pos >= lo) & (dst_pos < lo + NPAD)
    es = src_pos[sel]
    ed = dst_pos[sel] - lo
    order = np.argsort(ed, kind="stable")
    es, ed = es[order], ed[order]

    src_idx = np.zeros(EPAD, dtype=np.int16)
    bt = np.zeros((NL_T, CH, 128), dtype=np.float32)
    btT = np.zeros((NL_T, 128, CH), dtype=np.float32)
    tile_of = ed // 128
    starts = np.searchsorted(tile_of, np.arange(NL_T), side="left")
    ends = np.searchsorted(tile_of, np.arange(NL_T), side="right")
    for t in range(NL_T):
        p0 = t * CH
        # group 0: self loops of the tile's real nodes, in slot order
        nreal = int(tile_counts[core * NL_T + t])
        sl = np.arange(128)
        src_idx[p0:p0 + 128] = np.where(sl < nreal, lo + t * 128 + sl, 0)
        vd = sl[:nreal]
        bt[t, vd, vd] = 1.0
        btT[t, vd, vd] = 1.0
        # groups 1..: incident edges of this tile
        a, b = starts[t], ends[t]
        cnt = b - a
        assert 128 + cnt <= CH, f"dst tile {t}: {cnt} edges > {CH - 128}"
        src_idx[p0 + 128:p0 + 128 + cnt] = es[a:b].astype(np.int16)
        dl = (ed[a:b] - t * 128).astype(np.int64)
        bt[t, 128 + np.arange(cnt), dl] = 1.0
        btT[t, dl, 128 + np.arange(cnt)] = 1.0
    return src_idx, bt, btT


def wrap16(idx):
    """[EPAD] token array -> [128, EPAD/16] wrapped+replicated layout."""
    w = idx.reshape(-1, 16).T  # [16, EPAD/16]
    return np.ascontiguousarray(np.tile(w, (8, 1)))


def _bt_to_dram(bt, G):
    """bt [NL_T, CH, 128] (token-major) -> DRAM [NL_T, 128, CH] so that the
    SBUF tile [128, G, 128] slice [:, g, :] has token g*128+p at partition
    p: DRAM[k, p, g*128 + d] = bt[k, g*128 + p, d]."""
    NL_T, CH, _ = bt.shape
    out = bt.reshape(NL_T, G, 128, 128).transpose(0, 2, 1, 3)
    return np.ascontiguousarray(
        out.reshape(NL_T, 128, CH)).astype(ml_dtypes.bfloat16)


def host_prepare(cfg, x, pos, edge_index, batch,
                 W1, a_src1, a_dst1, b1, W2, a_src2, a_dst2, b2):
    PC, NG, NPAD, H, C, HC, FIN, B = (
        cfg["PC"], cfg["NG"], cfg["NPAD"], cfg["H"], cfg["C"],
        cfg["HC"], cfg["FIN"], cfg["B"])
    G = cfg["G"]
    NL_T = NPAD // 128
    ACT_W = HC + 2 * H
    NTBL = PC * NPAD
    bf = ml_dtypes.bfloat16

    x_in = np.concatenate([pos, x], axis=1).astype(np.float32)  # [NG, FIN]
    src = np.asarray(edge_index[0]).astype(np.int64)
    dst = np.asarray(edge_index[1]).astype(np.int64)

    # balance nodes across the 160 tiles by incident-edge count
    deg = np.bincount(dst, minlength=NG)
    npos, tile_counts = balance_nodes(deg, NTBL // 128)
    src_pos = npos[src]
    dst_pos = npos[dst]

    xpad = np.zeros((NTBL, FIN + 1), np.float32)
    xpad[:, FIN] = 1.0
    xpad[npos, 0:FIN] = x_in
    xt = np.ascontiguousarray(xpad.T).astype(bf)

    def augment(W, a_s, a_d, b):
        wad = np.einsum("fhc,hc->fh", W.reshape(W.shape[0], H, C), a_d)
        was = np.einsum("fhc,hc->fh", W.reshape(W.shape[0], H, C), a_s)
        waug = np.concatenate([W, was, wad], axis=1).astype(np.float32)
        cs = np.einsum("hc,hc->h", b.reshape(H, C), a_s)
        cd = np.einsum("hc,hc->h", b.reshape(H, C), a_d)
        brow = np.concatenate([b, cs, cd]).astype(np.float32)
        return waug, brow

    w1aug, b1row = augment(W1, a_src1, a_dst1, b1)
    w2aug, b2row = augment(W2, a_src2, a_dst2, b2)
    w1f = np.concatenate([w1aug, b1row[None, :]], axis=0).astype(bf)
    w2k = np.ascontiguousarray(
        w2aug.reshape(2, 128, ACT_W).transpose(1, 0, 2)).astype(bf)
    b2rv = b2row[None, :].astype(bf)
    ident = np.eye(128, dtype=np.float32)
    ones1 = np.ones((1, 128), dtype=bf)

    cnt = np.bincount(np.asarray(batch).astype(np.int64), minlength=B)
    wpos = 1.0 / np.maximum(cnt[np.asarray(batch).astype(np.int64)], 1.0)
    mp_full = np.zeros((NTBL, B), np.float32)
    mp_full[npos, np.asarray(batch).astype(np.int64)] = wpos
    in_maps = []
    for c in range(PC):
        si, bt, btT = pack_edges(cfg, src_pos, dst_pos, tile_counts, c)
        mp = mp_full[c * NPAD:(c + 1) * NPAD]
        mpool = np.ascontiguousarray(
            mp.reshape(NL_T, 128, B).transpose(1, 0, 2)).astype(bf)
        in_maps.append(dict(
            xt=xt, w1=w1f, w2=w2k, b2r=b2rv, ones1=ones1, ident=ident,
            srcw=wrap16(si),
            btd=_bt_to_dram(bt, G),
            btTd=np.ascontiguousarray(btT).astype(bf),
            mpool=mpool,
        ))
    return in_maps


def host_tail(pooled_parts, lw1, lb1, lw2, lb2):
    pooled = np.sum(np.stack(pooled_parts), axis=0)
    y = np.maximum(pooled @ lw1 + lb1, 0.0)
    y = np.maximum(y @ lw2 + lb2, 0.0)
    return y.astype(np.float32)


# ----------------------------------------------------------------------------
# entry point
# ----------------------------------------------------------------------------
_CACHE = {}


def kernel(**inputs):
    from concourse.bass_utils import run_bass_kernel_spmd

    cfg = full_cfg()
    inp = {k: np.asarray(v) for k, v in inputs.items()}
    in_maps = host_prepare(
        cfg, inp["x"], inp["pos"], inp["edge_index"], inp["batch"],
        inp["W1"], inp["a_src1"], inp["a_dst1"], inp["b1"],
        inp["W2"], inp["a_src2"], inp["a_dst2"], inp["b2"])
    if "nc" not in _CACHE:
        _CACHE["nc"] = build_program(cfg)
    nc = _CACHE["nc"]
    res = run_bass_kernel_spmd(nc, in_maps, list(range(cfg["PC"])))
    parts = [res.results[c]["pooled"] for c in range(cfg["PC"])]
    return host_tail(parts, inp["lw1"], inp["lb1"], inp["lw2"], inp["lb2"])
